# revision 6
# baseline (speedup 1.0000x reference)
"""GroupLinear (MoE routing) Trainium2 kernel.

Problem: x [8192, 1024] f32, indices [8192] int64 in [0,8),
W [8*2048, 1024] f32, b [8*2048] f32.
out[n] = x[n] @ W[g*2048:(g+1)*2048].T + b[g*2048:(g+1)*2048],  g = indices[n].

Strategy: expert-parallel across the 8 NeuronCores. Core g owns group g's
weight slice only (8MB instead of the full 64MB), and processes exactly the
rows routed to group g. Row routing (argsort of indices) happens on host;
the device kernel is a dense [C_pad, 1024] @ [1024, 2048] + bias matmul in
float32r (full PE rate, near-fp32 precision).

Host pre-layout puts both operands K-major *and* partition-major so every
DMA moves long contiguous lines per partition:
  x_r [128, 8*C_pad] : x_r[p, kc*C_pad + c] = x[rows[c], kc*128+p]
  w_r [128, 8*2048]  : w_r[p, kc*2048 + o]  = W_g[o, kc*128+p]
Loads go on the Sync HWDGE ring, stores + bias on the Scalar HWDGE ring so
store semaphore waits never block load issue. A junk-matmul warmup burst
lifts the PE HAM clock gate before the real matmuls arrive.
"""

import os
import sys

sys.path.insert(0, "/opt/trn_rl_repo")

import numpy as np

import concourse.bass as bass
import concourse.bacc as bacc
import concourse.mybir as mybir
import concourse.tile as tile
from concourse.bass_utils import run_bass_kernel_spmd

N = 8192
IN_F = 1024
OUT_F = 2048
G = 8
NCORES = 8
P = 128
NB_SZ = 512  # matmul moving-dim / PSUM bank free size (fp32)
N_WARMUP = 14  # junk matmuls to lift the PE clock gate during load phase

LAST_EXEC_NS = None
LAST_RESULTS = None

_nc_cache = {}


def _build_nc(c_pad: int):
    """Build the per-core Bass program for C_pad routed rows."""
    assert c_pad % P == 0
    kc_n = IN_F // P       # 8 k-chunks
    nb_n = OUT_F // NB_SZ  # 4 output-feature blocks
    mb_n = c_pad // P      # row blocks

    nc = bacc.Bacc("TRN2", target_bir_lowering=False, debug=False)
    f32r = mybir.dt.float32r

    x_r = nc.dram_tensor("x_r", [P, kc_n * c_pad], f32r, kind="ExternalInput")
    w_r = nc.dram_tensor("w_r", [P, kc_n * OUT_F], f32r, kind="ExternalInput")
    bias = nc.dram_tensor("bias", [1, OUT_F], mybir.dt.float32, kind="ExternalInput")
    out = nc.dram_tensor("out", [c_pad, OUT_F], mybir.dt.float32, kind="ExternalOutput")

    with tile.TileContext(nc) as tc:
        with (
            tc.tile_pool(name="wp", bufs=1) as wp,
            tc.tile_pool(name="xp", bufs=1) as xp,
            tc.tile_pool(name="bp", bufs=1) as bp,
            tc.tile_pool(name="op", bufs=3) as op,
            tc.tile_pool(name="pp", bufs=7, space="PSUM") as pp,
            tc.tile_pool(name="warm", bufs=1) as warmp,
            tc.tile_pool(name="warmps", bufs=1, space="PSUM") as warmpp,
        ):
            # -- PE warmup: junk matmuls with no data deps run immediately,
            # flipping the HAM clock gate to 2.4GHz while loads stream in.
            warm_sb = warmp.tile([P, NB_SZ], mybir.dt.bfloat16, name="warm_sb",
                                 tag="warm_sb")
            nc.vector.memset(warm_sb[:], 0.0)
            warm_ps = warmpp.tile([P, NB_SZ], mybir.dt.float32, name="warm_ps",
                                  tag="warm_ps")
            for i in range(N_WARMUP):
                nc.tensor.matmul(
                    warm_ps[:],
                    warm_sb[:, 0:P],
                    warm_sb[:],
                    start=(i == 0),
                    stop=(i == N_WARMUP - 1),
                )

            # bias broadcast on the scalar ring (keeps sync ring for loads)
            bias_sb = bp.tile([P, OUT_F], mybir.dt.float32, tag="bias")
            nc.scalar.dma_start(bias_sb[:], bias[0:1, :].to_broadcast((P, OUT_F)))

            # persistent x / w tiles, one per k-chunk; 4.6KB / 8KB lines
            x_sb = [None] * kc_n
            w_sb = [None] * kc_n
            for kc in range(kc_n):
                x_sb[kc] = xp.tile([P, c_pad], f32r, name=f"x{kc}", tag=f"x{kc}")
                nc.sync.dma_start(
                    x_sb[kc][:], x_r[:, kc * c_pad:(kc + 1) * c_pad]
                )
                w_sb[kc] = wp.tile([P, OUT_F], f32r, name=f"w{kc}", tag=f"w{kc}")
                nc.sync.dma_start(
                    w_sb[kc][:], w_r[:, kc * OUT_F:(kc + 1) * OUT_F]
                )

            for mb in range(mb_n):
                psums = []
                for nb in range(nb_n):
                    psums.append(
                        pp.tile([P, NB_SZ], mybir.dt.float32, name=f"ps{mb}_{nb}",
                                tag="psum")
                    )
                for kc in range(kc_n):
                    lhsT = x_sb[kc][:, mb * P:(mb + 1) * P]
                    for nb in range(nb_n):
                        nc.tensor.matmul(
                            psums[nb][:],
                            lhsT,
                            w_sb[kc][:, nb * NB_SZ:(nb + 1) * NB_SZ],
                            start=(kc == 0),
                            stop=(kc == kc_n - 1),
                        )
                orow = op.tile([P, OUT_F], mybir.dt.float32, name=f"orow{mb}",
                               tag="orow")
                for nb in range(nb_n):
                    nc.vector.tensor_add(
                        orow[:, nb * NB_SZ:(nb + 1) * NB_SZ],
                        psums[nb][:],
                        bias_sb[:, nb * NB_SZ:(nb + 1) * NB_SZ],
                    )
                nc.scalar.dma_start(out[mb * P:(mb + 1) * P, :], orow[:])

    nc.compile()
    return nc


def _get_nc(c_pad: int):
    nc = _nc_cache.get(c_pad)
    if nc is None:
        nc = _build_nc(c_pad)
        _nc_cache[c_pad] = nc
    return nc


def kernel(x, indices, W, b):
    global LAST_EXEC_NS, LAST_RESULTS

    x = np.ascontiguousarray(np.asarray(x, dtype=np.float32))
    W = np.ascontiguousarray(np.asarray(W, dtype=np.float32))
    b = np.asarray(b, dtype=np.float32)
    idx = np.asarray(indices).astype(np.int64)

    order = np.argsort(idx, kind="stable")
    counts = np.bincount(idx, minlength=G)
    offs = np.zeros(G + 1, dtype=np.int64)
    np.cumsum(counts, out=offs[1:])

    c_pad = max(P, int(-(-counts.max() // P)) * P)
    kc_n = IN_F // P
    nc = _get_nc(c_pad)

    rows = [order[offs[g]:offs[g + 1]] for g in range(G)]
    in_maps = []
    for g in range(G):
        # xT [1024, c_pad] -> x_r [128, 8*c_pad] (partition-major k-chunks)
        xT = np.zeros((IN_F, c_pad), dtype=np.float32)
        cg = int(counts[g])
        if cg:
            xT[:, :cg] = x[rows[g]].T
        xr = np.ascontiguousarray(
            xT.reshape(kc_n, P, c_pad).swapaxes(0, 1).reshape(P, kc_n * c_pad)
        )
        wT = W[g * OUT_F:(g + 1) * OUT_F, :].T  # [1024, 2048]
        wr = np.ascontiguousarray(
            wT.reshape(kc_n, P, OUT_F).swapaxes(0, 1).reshape(P, kc_n * OUT_F)
        )
        bg = np.ascontiguousarray(b[g * OUT_F:(g + 1) * OUT_F]).reshape(1, OUT_F)
        in_maps.append({"x_r": xr, "w_r": wr, "bias": bg})

    trace = bool(int(os.environ.get("KERNEL_TRACE", "0")))
    res = run_bass_kernel_spmd(nc, in_maps, list(range(NCORES)), trace=trace)
    LAST_EXEC_NS = res.exec_time_ns
    LAST_RESULTS = res

    out = np.empty((N, OUT_F), dtype=np.float32)
    for g in range(G):
        cg = int(counts[g])
        if cg:
            out[rows[g]] = res.results[g]["out"][:cg]
    return out


# revision 10
# speedup vs baseline: 1.1464x; 1.1464x over previous
"""GroupLinear (MoE routing) Trainium2 kernel.

Problem: x [8192, 1024] f32, indices [8192] int64 in [0,8),
W [8*2048, 1024] f32, b [8*2048] f32.
out[n] = x[n] @ W[g*2048:(g+1)*2048].T + b[g*2048:(g+1)*2048],  g = indices[n].

Strategy: expert-parallel across the 8 NeuronCores. Core g owns group g's
weight slice only (8MB instead of the full 64MB), and processes exactly the
rows routed to group g. Row routing (argsort of indices) happens on host;
the device kernel is a dense [C_pad, 1024] @ [1024, 2048] + bias matmul in
float32r (full PE rate, near-fp32 precision).

Host pre-layout puts both operands K-major *and* partition-major so every
DMA moves long contiguous lines per partition:
  x_r [128, 8*C_pad] : x_r[p, kc*C_pad + c] = x[rows[c], kc*128+p]
  w_r [128, 8*2048]  : w_r[p, kc*2048 + o]  = W_g[o, kc*128+p]
Loads go on the Sync HWDGE ring, stores + bias on the Scalar HWDGE ring so
store semaphore waits never block load issue. A junk-matmul warmup burst
lifts the PE HAM clock gate before the real matmuls arrive.
"""

import os
import sys

sys.path.insert(0, "/opt/trn_rl_repo")

import numpy as np

import concourse.bass as bass
import concourse.bacc as bacc
import concourse.mybir as mybir
import concourse.tile as tile
from concourse.bass_utils import run_bass_kernel_spmd

N = 8192
IN_F = 1024
OUT_F = 2048
G = 8
NCORES = 8
P = 128
NB_SZ = 512  # matmul moving-dim / PSUM bank free size (fp32)
N_WARMUP = 14  # junk matmuls to lift the PE clock gate during load phase

LAST_EXEC_NS = None
LAST_RESULTS = None

_nc_cache = {}


def _build_nc(c_pad: int):
    """Build the per-core Bass program for C_pad routed rows."""
    assert c_pad % P == 0
    kc_n = IN_F // P       # 8 k-chunks
    nb_n = OUT_F // NB_SZ  # 4 output-feature blocks
    mb_n = c_pad // P      # row blocks

    nc = bacc.Bacc("TRN2", target_bir_lowering=False, debug=False)
    f32r = mybir.dt.float32r

    x_r = nc.dram_tensor("x_r", [P, c_pad * IN_F // P], f32r, kind="ExternalInput")
    w_r = nc.dram_tensor("w_r", [P, kc_n * OUT_F], f32r, kind="ExternalInput")
    bias = nc.dram_tensor("bias", [1, OUT_F], mybir.dt.float32, kind="ExternalInput")
    out = nc.dram_tensor("out", [c_pad, OUT_F], mybir.dt.float32, kind="ExternalOutput")

    with tile.TileContext(nc) as tc:
        with (
            tc.tile_pool(name="wp", bufs=1) as wp,
            tc.tile_pool(name="xp", bufs=1) as xp,
            tc.tile_pool(name="bp", bufs=1) as bp,
            tc.tile_pool(name="op", bufs=6) as op,
            tc.tile_pool(name="pp", bufs=7, space="PSUM") as pp,
            tc.tile_pool(name="warm", bufs=1) as warmp,
            tc.tile_pool(name="warmps", bufs=1, space="PSUM") as warmpp,
        ):
            # -- PE warmup: junk matmuls with no data deps run immediately,
            # flipping the HAM clock gate to 2.4GHz while loads stream in.
            warm_sb = warmp.tile([P, NB_SZ], mybir.dt.bfloat16, name="warm_sb",
                                 tag="warm_sb")
            nc.vector.memset(warm_sb[:], 0.0)
            warm_ps = warmpp.tile([P, NB_SZ], mybir.dt.float32, name="warm_ps",
                                  tag="warm_ps")
            for i in range(N_WARMUP):
                nc.tensor.matmul(
                    warm_ps[:],
                    warm_sb[:, 0:P],
                    warm_sb[:],
                    start=(i == 0),
                    stop=(i == N_WARMUP - 1),
                )

            # bias broadcast on the scalar ring (keeps sync ring for loads)
            bias_sb = bp.tile([P, OUT_F], mybir.dt.float32, tag="bias")
            nc.scalar.dma_start(bias_sb[:], bias[0:1, :].to_broadcast((P, OUT_F)))

            # x: one [128, 1024] piece per row-block (4KB lines), covering
            # all k-chunks; w: one [128, 4096] piece per output column
            # (16KB lines), covering all k-chunks. First column starts
            # after x_mb0 + w_nb0 = 2.5MB.
            x_sb = [None] * mb_n
            w_sb = [None] * nb_n
            w_sb[0] = wp.tile([P, kc_n * NB_SZ], f32r, name="w0", tag="w0")
            nc.sync.dma_start(w_sb[0][:], w_r[:, 0:kc_n * NB_SZ])
            for mb in range(mb_n):
                x_sb[mb] = xp.tile([P, IN_F], f32r, name=f"x{mb}", tag=f"x{mb}")
                nc.sync.dma_start(
                    x_sb[mb][:], x_r[:, mb * IN_F:(mb + 1) * IN_F]
                )
            for nb in range(1, nb_n):
                w_sb[nb] = wp.tile([P, kc_n * NB_SZ], f32r, name=f"w{nb}",
                                   tag=f"w{nb}")
                nc.sync.dma_start(
                    w_sb[nb][:],
                    w_r[:, nb * kc_n * NB_SZ:(nb + 1) * kc_n * NB_SZ],
                )

            for nb in range(nb_n):
                for mb in range(mb_n):
                    psum = pp.tile([P, NB_SZ], mybir.dt.float32,
                                   name=f"ps{nb}_{mb}", tag="psum")
                    for kc in range(kc_n):
                        nc.tensor.matmul(
                            psum[:],
                            x_sb[mb][:, kc * P:(kc + 1) * P],
                            w_sb[nb][:, kc * NB_SZ:(kc + 1) * NB_SZ],
                            start=(kc == 0),
                            stop=(kc == kc_n - 1),
                        )
                    ot = op.tile([P, NB_SZ], mybir.dt.float32,
                                 name=f"ot{nb}_{mb}", tag="ot")
                    nc.vector.tensor_add(
                        ot[:], psum[:], bias_sb[:, nb * NB_SZ:(nb + 1) * NB_SZ]
                    )
                    nc.scalar.dma_start(
                        out[mb * P:(mb + 1) * P, nb * NB_SZ:(nb + 1) * NB_SZ],
                        ot[:],
                    )

    nc.compile()
    return nc


def _get_nc(c_pad: int):
    nc = _nc_cache.get(c_pad)
    if nc is None:
        nc = _build_nc(c_pad)
        _nc_cache[c_pad] = nc
    return nc


def kernel(x, indices, W, b):
    global LAST_EXEC_NS, LAST_RESULTS

    x = np.ascontiguousarray(np.asarray(x, dtype=np.float32))
    W = np.ascontiguousarray(np.asarray(W, dtype=np.float32))
    b = np.asarray(b, dtype=np.float32)
    idx = np.asarray(indices).astype(np.int64)

    order = np.argsort(idx, kind="stable")
    counts = np.bincount(idx, minlength=G)
    offs = np.zeros(G + 1, dtype=np.int64)
    np.cumsum(counts, out=offs[1:])

    c_pad = max(P, int(-(-counts.max() // P)) * P)
    kc_n = IN_F // P
    nc = _get_nc(c_pad)

    rows = [order[offs[g]:offs[g + 1]] for g in range(G)]
    mb_n = c_pad // P
    nb_n = OUT_F // NB_SZ
    in_maps = []
    for g in range(G):
        # x_r [128, mb_n*1024]: piece mb holds x_r[p, mb*1024 + kc*128 + c]
        #   = x[rows[mb*128+c], kc*128+p]
        xT = np.zeros((IN_F, c_pad), dtype=np.float32)
        cg = int(counts[g])
        if cg:
            xT[:, :cg] = x[rows[g]].T
        xr = np.ascontiguousarray(
            xT.reshape(kc_n, P, mb_n, P)
            .transpose(1, 2, 0, 3)
            .reshape(P, mb_n * IN_F)
        )
        # w_r [128, nb_n*8*512]: piece nb holds w_r[p, nb*4096 + kc*512 + o]
        #   = W_g[nb*512+o, kc*128+p]
        wT = W[g * OUT_F:(g + 1) * OUT_F, :].T  # [1024, 2048]
        wr = np.ascontiguousarray(
            wT.reshape(kc_n, P, nb_n, NB_SZ)
            .transpose(1, 2, 0, 3)
            .reshape(P, kc_n * OUT_F)
        )
        bg = np.ascontiguousarray(b[g * OUT_F:(g + 1) * OUT_F]).reshape(1, OUT_F)
        in_maps.append({"x_r": xr, "w_r": wr, "bias": bg})

    trace = bool(int(os.environ.get("KERNEL_TRACE", "0")))
    res = run_bass_kernel_spmd(nc, in_maps, list(range(NCORES)), trace=trace)
    LAST_EXEC_NS = res.exec_time_ns
    LAST_RESULTS = res

    out = np.empty((N, OUT_F), dtype=np.float32)
    for g in range(G):
        cg = int(counts[g])
        if cg:
            out[rows[g]] = res.results[g]["out"][:cg]
    return out
